# revision 29
# baseline (speedup 1.0000x reference)
"""Distributed Trainium2 kernel for ViTDet-style global attention with
decomposed relative position bias (B=1, H=W=64, C=768, 12 heads, hd=64).

Sharding: 4 head-groups x 2 query-halves over 8 cores. Core r handles
heads 3*hg..3*hg+2 (hg = r % 4) and queries qh*2048..qh*2048+2047
(qh = r // 4). Each core computes K/V for its 3 heads over all 4096
keys, attention for its 2048 queries, and a PARTIAL output projection
contracting only its 192 channels; the host sums the 4 partials per
query half. No collectives.

Inside a core the 3x2048 work is organized as 12 "virtual heads"
vh = qs*3 + h of (512-query subtile, head), reusing the proven
single-core structure: rel_h folded into the QK matmul via one-hot
indicator rows on an augmented K, rel_w applied as a post-exp
multiplicative factor (its partition pattern has period 64, matching
k mod 64), softmax without max subtraction, PV with a ones-column for
the denominator.

Scheduling: a minimal pre-attention phase (K heads 0/1, V heads 0/1,
q-projection + rel tables for query-subtile 0) starts attention ~30us
in; K head 2, V head 2, the qs1-3 q/rel chains, softmax denominators,
and the output projection are all deferred and pumped through the
attention phase's PE slack via a deadline-tagged fill queue, so the
Act engine (exp) runs back-to-back. Startup DMAs are split across the
SP and Act DMA queues.
"""

import sys

import numpy as np
import ml_dtypes

for p in ("/opt/trn_rl_repo",):
    if p not in sys.path:
        sys.path.insert(0, p)

import concourse.mybir as mybir
from concourse import bacc
from concourse.tile import TileContext
from concourse.bass_utils import run_bass_kernel_spmd

NCORES = 8
S, C, NH, HD = 4096, 768, 12, 64
NHG = 4                   # head groups
NQH = 2                   # query halves
HPC = NH // NHG           # 3 heads per core
SQH = S // NQH            # 2048 queries per core
NVH = HPC * 4             # 12 virtual heads (512-q subtile, head)
SQ = 512                  # queries per virtual head
HLOC = SQH // 64          # 32 h-rows per core
BLK = 512                 # x block (keys)
NBLK = S // BLK           # 8
NKC = S // 128            # 32 key chunks
EBATCH = 2                # logits chunks per exp batch (2 PSUM banks)
CH = HPC * HD             # 192 channels per core
F32 = mybir.dt.float32
BF = mybir.dt.bfloat16
Exp = mybir.ActivationFunctionType.Exp
Ident = mybir.ActivationFunctionType.Identity
Mult = mybir.AluOpType.mult
BF_NP = ml_dtypes.bfloat16

LAST_EXEC_NS = None
DEBUG_DUMPS = False


def build():
    nc = bacc.Bacc(None, target_bir_lowering=False)

    xT = nc.dram_tensor("xT", (6, 128, S), BF, kind="ExternalInput")
    wq = nc.dram_tensor("wq", (128, 6, CH), BF, kind="ExternalInput")
    wk = nc.dram_tensor("wk", (128, 6, CH), BF, kind="ExternalInput")
    wv = nc.dram_tensor("wv", (128, 6, CH), BF, kind="ExternalInput")
    pwA = nc.dram_tensor("pwA", (128, 6, 128), BF, kind="ExternalInput")
    pwB = nc.dram_tensor("pwB", (64, 6, 128), BF, kind="ExternalInput")
    bq = nc.dram_tensor("bq", (64, HPC), F32, kind="ExternalInput")
    pb2 = nc.dram_tensor("pb2", (128, 6), F32, kind="ExternalInput")
    rhT = nc.dram_tensor("rhT", (HD, HLOC * 64), BF, kind="ExternalInput")
    rwT = nc.dram_tensor("rwT", (HD, 64 * 64), BF, kind="ExternalInput")
    idm = nc.dram_tensor("idm", (128, 128), BF, kind="ExternalInput")
    khfull = nc.dram_tensor("khfull", (64, S), BF, kind="ExternalInput")
    out = nc.dram_tensor("out", (C, SQH), F32, kind="ExternalOutput")

    with TileContext(nc) as tc:
        with (
            nc.allow_low_precision(reason="bf16 matmul inputs"),
            tc.tile_pool(name="per", bufs=1) as per,
            tc.tile_pool(name="pt", bufs=6) as ptp,
            tc.tile_pool(name="ysb", bufs=2) as ysbp,
            tc.tile_pool(name="sml", bufs=1) as smlp,
            tc.tile_pool(name="stp", bufs=3, space="PSUM") as stp,
            tc.tile_pool(name="pvp", bufs=2, space="PSUM") as pvp,
        ):
            # ---- DMAs on the SP queue: pb, K/V weights, x blocks ----
            pb_sb = per.tile([128, 6], F32, tag="pbsb")
            nc.sync.dma_start(pb_sb[:], pb2[:])
            idt = per.tile([128, 128], BF, tag="idt")
            nc.sync.dma_start(idt[:], idm[:])
            wk_sb = per.tile([128, 6, CH], BF, tag="wk")
            nc.sync.dma_start(wk_sb[:], wk[:])
            wv_sb = per.tile([128, 6, CH], BF, tag="wv")
            nc.sync.dma_start(wv_sb[:], wv[:])
            xall = per.tile([128, 6, S], BF, tag="xall")
            for blk in range(NBLK):
                sl = slice(blk * BLK, (blk + 1) * BLK)
                nc.sync.dma_start(
                    xall[:, :, sl], xT[:, :, sl].transpose([1, 0, 2]))

            # ---- DMAs on the Act queue (parallel with the above) ----
            bq_sb = per.tile([64, HPC], F32, tag="bq")
            nc.scalar.dma_start(bq_sb[:], bq[:])
            wq_sb = per.tile([128, 6, CH], BF, tag="wq")
            nc.scalar.dma_start(wq_sb[:], wq[:])
            rhT_sb = per.tile([HD, HLOC * 64], BF, tag="rhT")
            nc.scalar.dma_start(rhT_sb[:], rhT[:])
            rwT_sb = per.tile([HD, 64 * 64], BF, tag="rwT")
            nc.scalar.dma_start(rwT_sb[:], rwT[:])

            # ---- persistent tiles ----
            kts = [per.tile([128, S], BF, tag=f"kt{i}", name=f"kt{i}")
                   for i in range(HPC)]
            for i in range(HPC):
                nc.scalar.dma_start(kts[i][64:128, :], khfull[:])
            pwA_sb = per.tile([128, 6, 128], BF, tag="pwA")
            nc.scalar.dma_start(pwA_sb[:], pwA[:])
            pwB_sb = per.tile([64, 6, 128], BF, tag="pwB")
            nc.scalar.dma_start(pwB_sb[:], pwB[:])

            vtA = per.tile([128, NKC, HPC, 66], BF, tag="vtA")
            nc.vector.memset(vtA[:, :, :, 64], 1.0)
            qaT = per.tile([128, NVH, SQ], BF, tag="qaT")
            relwA = per.tile([64, NVH, SQ], BF, tag="relwA")
            ewA = per.tile([128, NVH, SQ], BF, tag="ewA")
            aout0 = per.tile([128, SQH], BF, tag="ao0")
            aout1 = per.tile([64, SQH], BF, tag="ao1")

            # ---- emit helpers ----
            def kpair_blk(blk):
                # K for heads 0,1 packed on 128 partitions
                sl = slice(blk * BLK, (blk + 1) * BLK)
                psk0 = stp.tile([128, 2, 512], F32, tag="st")
                psk = psk0[:, 0, :]
                for cc in range(6):
                    nc.tensor.matmul(
                        psk[:], wk_sb[:, cc, 0:128], xall[:, cc, sl],
                        start=(cc == 0), stop=(cc == 5),
                        skip_group_check=True)
                nc.scalar.copy(kts[0][0:64, sl], psk[0:64, :])
                nc.vector.tensor_copy(kts[1][0:64, sl], psk[64:128, :])

            def v01_blk(blk):
                # V for heads 0,1 per 128-key chunk
                for rb in range(4):
                    kc = 4 * blk + rb
                    ksl = slice(blk * BLK + rb * 128,
                                blk * BLK + (rb + 1) * 128)
                    psv0 = stp.tile([128, 16, 64], F32, tag="st")
                    psv = psv0[:, 0:2, :]
                    for cc in range(6):
                        nc.tensor.matmul(
                            psv[:], xall[:, cc, ksl], wv_sb[:, cc, 0:128],
                            start=(cc == 0), stop=(cc == 5),
                            skip_group_check=True)
                    nc.vector.tensor_copy(vtA[:, kc, 0:2, 0:64], psv[:])

            def k2_blk(blk):
                # K head 2, one 512-key block ([64,512] out) per item
                sl = slice(blk * BLK, (blk + 1) * BLK)
                ps0 = stp.tile([128, 2, 512], F32, tag="st")
                ps2 = ps0[0:64, 0, :]
                for cc in range(6):
                    nc.tensor.matmul(
                        ps2[:], wk_sb[:, cc, 128:192], xall[:, cc, sl],
                        start=(cc == 0), stop=(cc == 5),
                        skip_group_check=True)
                nc.vector.tensor_copy(kts[2][0:64, sl], ps2[:])

            def v2_rb(kc):
                ksl = slice(kc * 128, (kc + 1) * 128)
                psv0 = stp.tile([128, 16, 64], F32, tag="st")
                psv = psv0[:, 0, :]
                for cc in range(6):
                    nc.tensor.matmul(
                        psv[:], xall[:, cc, ksl], wv_sb[:, cc, 128:192],
                        start=(cc == 0), stop=(cc == 5),
                        skip_group_check=True)
                nc.vector.tensor_copy(vtA[:, kc, 2, 0:64], psv[:])

            def qproj(qs):
                # heads 0,1 packed + head 2; bias folded into the copies.
                # Queries are xall columns 0:2048 (keys pre-rolled per
                # core so each core's query window sits at offset 0).
                ps0 = stp.tile([128, 2, 512], F32, tag="st")
                ps = ps0[:, 0, :]
                qsl = slice(qs * 512, (qs + 1) * 512)
                for cc in range(6):
                    nc.tensor.matmul(
                        ps[:], wq_sb[:, cc, 0:128], xall[:, cc, qsl],
                        start=(cc == 0), stop=(cc == 5),
                        skip_group_check=True)
                ps2 = ps0[0:64, 1, :]
                for cc in range(6):
                    nc.tensor.matmul(
                        ps2[:], wq_sb[:, cc, 128:192], xall[:, cc, qsl],
                        start=(cc == 0), stop=(cc == 5),
                        skip_group_check=True)
                for hh in range(2):
                    nc.scalar.activation(
                        qaT[0:64, qs * 3 + hh, :],
                        ps[hh * 64:hh * 64 + 64, :],
                        Ident, bias=bq_sb[:, hh:hh + 1])
                nc.scalar.activation(
                    qaT[0:64, qs * 3 + 2, :], ps2[:],
                    Ident, bias=bq_sb[:, 2:3])

            def relh(b):
                qs = b // 3
                ps0 = stp.tile([128, 2, 512], F32, tag="st")
                ps = ps0[0:64, 0, :]
                for hl in range(8):
                    lr = qs * 8 + hl
                    nc.tensor.matmul(
                        ps[:, hl * 64:(hl + 1) * 64],
                        rhT_sb[:, lr * 64:(lr + 1) * 64],
                        qaT[0:64, b, hl * 64:(hl + 1) * 64],
                        start=True, stop=True, skip_group_check=True)
                nc.vector.tensor_copy(qaT[64:128, b, :], ps[:])

            def relw(wv0, n):
                # one matmul per w-coordinate covers all 12 virtual heads
                for wvi in range(wv0, wv0 + n):
                    ps0 = stp.tile([128, 4, 12, 8], F32, tag="st")
                    ps = ps0[0:64, 0, :, :]
                    nc.tensor.matmul(
                        ps[:],
                        rwT_sb[:, wvi * 64:(wvi + 1) * 64],
                        qaT[0:64, :, wvi:wvi + 449:64],
                        start=True, stop=True, skip_group_check=True)
                    if wvi % 2 == 0:
                        nc.vector.tensor_copy(
                            relwA[:, :, wvi:wvi + 449:64], ps[:])
                    else:
                        nc.scalar.copy(
                            relwA[:, :, wvi:wvi + 449:64], ps[:])

            def ew_exp():
                nc.scalar.activation(ewA[0:64, :, :], relwA[:], Exp)
                nc.gpsimd.tensor_copy(ewA[64:128, :, :], ewA[0:64, :, :])

            def out_proj(t, qs):
                ps0 = stp.tile([128, 2, 512], F32, tag="st")
                ps = ps0[:, 0, :]
                sl = slice(qs * 512, (qs + 1) * 512)
                nc.tensor.matmul(
                    ps[:], pwA_sb[:, t, :], aout0[:, sl],
                    start=True, stop=False, skip_group_check=True)
                nc.tensor.matmul(
                    ps[:], pwB_sb[:, t, :], aout1[:, sl],
                    start=False, stop=True, skip_group_check=True)
                y = ysbp.tile([128, 512], F32, tag="y")
                nc.vector.tensor_scalar_add(y[:], ps[:], pb_sb[:, t:t + 1])
                nc.sync.dma_start(out[t * 128:(t + 1) * 128, sl], y[:])

            # ---- pre-attention phase ----
            for blk in range(NBLK):
                kpair_blk(blk)
                v01_blk(blk)
                if blk >= 1 and blk <= 4:
                    qproj(blk - 1)
                if blk >= 5:
                    relw((blk - 5) * 16, 16)
            relw(48, 16)
            for b in range(3):
                relh(b)
            ew_exp()

            # ---- deferred work: deadline-tagged fill queue ----
            fill_q = []

            def enqueue(deadline, fn):
                fill_q.append((deadline, fn))

            for blk in range(NBLK):
                enqueue(2, lambda blk=blk: k2_blk(blk))
                enqueue(2, lambda kc=4 * blk: v2_rb(kc))
                enqueue(2, lambda kc=4 * blk + 1: v2_rb(kc))
                enqueue(2, lambda kc=4 * blk + 2: v2_rb(kc))
                enqueue(2, lambda kc=4 * blk + 3: v2_rb(kc))
            for b in range(3, NVH):
                enqueue(b, lambda b=b: relh(b))

            def pump(n):
                for _ in range(n):
                    if fill_q:
                        fill_q.pop(0)[1]()

            def drain_until(dl):
                while fill_q and fill_q[0][0] <= dl:
                    fill_q.pop(0)[1]()

            # ---- attention, Act-saturated pipeline ----
            batches = []
            kc = 0
            while kc < NKC:
                nb = min(EBATCH, NKC - kc)
                batches.append((kc, nb))
                kc += nb
            NB = len(batches)

            def qk_batch(b, kc, nb):
                st = stp.tile([128, 2, 512], F32, tag="st")
                kt = kts[b % 3]
                for sub in range(nb):
                    nc.tensor.matmul(
                        st[:, sub, :],
                        kt[:, (kc + sub) * 128:(kc + sub + 1) * 128],
                        qaT[:, b, :],
                        start=True, stop=True, skip_group_check=True)
                return st

            def make_denoms(b, pv):
                # 5 thunks: reciprocal, then per-qc scale+transpose+copy;
                # the last thunk enqueues the output projections that
                # become ready once this vh's aout columns are complete.
                head, qs = b % 3, b // 3
                state = {}

                def d_recip():
                    r = smlp.tile([128, 4, 1], F32, tag="recip", bufs=2)
                    nc.vector.reciprocal(r[:], pv[:, :, 64:65])
                    state["r"] = r

                def d_qc(qc):
                    aoT = smlp.tile([128, 64], BF, tag="aoT", bufs=2)
                    nc.vector.tensor_scalar_mul(
                        aoT[:], pv[:, qc, 0:64], state["r"][:, qc, :])
                    tps = pv[0:64, qc, 64:128].bitcast(BF)
                    nc.tensor.transpose(tps, aoT[:], idt[:])
                    dst_q = qs * 512 + qc * 128
                    if head < 2:
                        nc.vector.tensor_copy(
                            aout0[head * 64:head * 64 + 64,
                                  dst_q:dst_q + 128], tps)
                    else:
                        nc.vector.tensor_copy(
                            aout1[0:64, dst_q:dst_q + 128], tps)
                    if head == 2 and qc == 3:
                        for t in range(6):
                            enqueue(99, lambda t=t, qs=qs: out_proj(t, qs))

                return [d_recip] + [lambda qc=qc: d_qc(qc)
                                    for qc in range(4)]

            st = qk_batch(0, *batches[0])
            denom_prev = []
            for b in range(NVH):
                pv = pvp.tile([128, 4, 128], F32, tag="pv")
                hh = b % 3
                for bi, (kc, nb) in enumerate(batches):
                    pt = ptp.tile([128, 2, 512], BF, tag="pt")
                    nc.scalar.activation(
                        pt[:, 0:nb, :], st[:, 0:nb, :], Exp)
                    meng = nc.gpsimd if bi % 3 == 2 else nc.vector
                    meng.tensor_tensor(
                        pt[:, 0:nb, :], pt[:, 0:nb, :],
                        ewA[:, b, :].unsqueeze(1).to_broadcast(
                            (128, nb, 512)),
                        Mult)
                    # next QK before this PV so PE stays a batch ahead
                    if bi + 1 < NB:
                        st = qk_batch(b, *batches[bi + 1])
                    elif b + 1 < NVH:
                        drain_until(b + 1)
                        st = qk_batch(b + 1, *batches[0])
                    for sub in range(nb):
                        for qc in range(4):
                            nc.tensor.matmul(
                                pv[:, qc, 0:65],
                                pt[:, sub, qc * 128:(qc + 1) * 128],
                                vtA[:, kc + sub, hh, 0:65],
                                start=(kc + sub == 0 and qc == 0),
                                stop=(kc + sub == NKC - 1 and qc == 3),
                                skip_group_check=True)
                    if bi < len(denom_prev):
                        denom_prev[bi]()
                    pump(2 if len(fill_q) > 24 else 1)
                denom_prev = make_denoms(b, pv)

            for fn in denom_prev:
                fn()
            pump(len(fill_q))

            if DEBUG_DUMPS:
                d_qaT = nc.dram_tensor(
                    "d_qaT", (128, NVH, SQ), BF, kind="ExternalOutput")
                nc.sync.dma_start(d_qaT[:], qaT[:])
                d_kt0 = nc.dram_tensor(
                    "d_kt0", (128, S), BF, kind="ExternalOutput")
                nc.sync.dma_start(d_kt0[:], kts[0][:])
                d_vt = nc.dram_tensor(
                    "d_vt", (128, NKC, HPC, 65), BF, kind="ExternalOutput")
                nc.sync.dma_start(d_vt[:], vtA[:, :, :, 0:65])
                d_ew = nc.dram_tensor(
                    "d_ew", (128, NVH, SQ), BF, kind="ExternalOutput")
                nc.sync.dma_start(d_ew[:], ewA[:])
                d_ao0 = nc.dram_tensor(
                    "d_ao0", (128, SQH), BF, kind="ExternalOutput")
                nc.sync.dma_start(d_ao0[:], aout0[:])
                d_ao1 = nc.dram_tensor(
                    "d_ao1", (64, SQH), BF, kind="ExternalOutput")
                nc.sync.dma_start(d_ao1[:], aout1[:])

    nc.compile()
    return nc


def prepare_in_maps(x, qkv_w, qkv_b, proj_w, proj_b, rel_pos_h, rel_pos_w):
    x = np.asarray(x, np.float32)
    qkv_w = np.asarray(qkv_w, np.float32)
    qkv_b = np.asarray(qkv_b, np.float32)
    proj_w = np.asarray(proj_w, np.float32)
    proj_b = np.asarray(proj_b, np.float32)
    rel_pos_h = np.asarray(rel_pos_h, np.float32)
    rel_pos_w = np.asarray(rel_pos_w, np.float32)

    scale = HD ** -0.5
    xT = np.ascontiguousarray(x.reshape(S, C).T)          # (768, 4096)
    xTr = xT.reshape(6, 128, S).astype(BF_NP)

    w_q_full = (qkv_w[:, :C] * scale).reshape(6, 128, C)
    w_k_full = qkv_w[:, C:2 * C].reshape(6, 128, C)
    w_v_full = qkv_w[:, 2 * C:].reshape(6, 128, C)
    bq_full = (qkv_b[:C] * scale)
    bv = qkv_b[2 * C:]

    k_idx = np.arange(64)
    rwT = np.empty((HD, 64, 64), np.float32)
    for wvi in range(64):
        rwT[:, wvi, :] = rel_pos_w[wvi + 63 - k_idx, :].T / scale
    rwT = rwT.reshape(HD, 64 * 64).astype(BF_NP)

    idmn = np.eye(128, dtype=np.float32).astype(BF_NP)
    khfull = ((np.arange(S)[None, :] // 64 == np.arange(64)[:, None])
              .astype(BF_NP))
    # Per-query-half KEY ROLL: softmax is permutation-invariant over
    # keys, and the rel_w partition pattern has period 64 (2048 % 64
    # == 0), so rolling the key axis by -qh*2048 lets every core read
    # its own query window at xall columns 0:2048. khfull (the h-row
    # indicator) must be rolled consistently.
    xTr_r = [np.roll(xTr, -qh * SQH, axis=2) for qh in range(NQH)]
    khfull_r = [np.roll(khfull, -qh * SQH, axis=1) for qh in range(NQH)]

    rhTs = []
    for qh in range(NQH):
        rhm = np.empty((HD, HLOC, 64), np.float32)
        for lr in range(HLOC):
            h = qh * HLOC + lr
            rhm[:, lr, :] = rel_pos_h[h + 63 - k_idx, :].T / scale
        rhTs.append(rhm.reshape(HD, HLOC * 64).astype(BF_NP))

    wqs, wks, wvs, pwAs, pwBs, bqs, pb2s = [], [], [], [], [], [], []
    for hg in range(NHG):
        cs = slice(hg * CH, (hg + 1) * CH)
        wqs.append(np.ascontiguousarray(
            w_q_full[:, :, cs].transpose(1, 0, 2)).astype(BF_NP))
        wks.append(np.ascontiguousarray(
            w_k_full[:, :, cs].transpose(1, 0, 2)).astype(BF_NP))
        wvs.append(np.ascontiguousarray(
            w_v_full[:, :, cs].transpose(1, 0, 2)).astype(BF_NP))
        pw_slice = proj_w[hg * CH:(hg + 1) * CH, :]     # (192, 768)
        pwAs.append(np.ascontiguousarray(
            pw_slice[0:128].reshape(128, 6, 128)).astype(BF_NP))
        pwBs.append(np.ascontiguousarray(
            pw_slice[128:192].reshape(64, 6, 128)).astype(BF_NP))
        bqs.append(np.ascontiguousarray(
            bq_full[cs].reshape(HPC, 64).T).astype(np.float32))
        pb_r = bv[cs] @ pw_slice
        if hg == 0:
            pb_r = pb_r + proj_b
        pb2s.append(np.ascontiguousarray(
            pb_r.reshape(6, 128).T).astype(np.float32))

    in_maps = []
    for r in range(NCORES):
        hg, qh = r % NHG, r // NHG
        in_maps.append({
            "xT": xTr_r[qh], "wq": wqs[hg], "wk": wks[hg],
            "wv": wvs[hg], "pwA": pwAs[hg], "pwB": pwBs[hg],
            "bq": bqs[hg], "pb2": pb2s[hg], "rhT": rhTs[qh], "rwT": rwT,
            "idm": idmn, "khfull": khfull_r[qh],
        })
    return in_maps


_NC = None


def kernel(x, qkv_w, qkv_b, proj_w, proj_b, rel_pos_h, rel_pos_w):
    global _NC, LAST_EXEC_NS
    in_maps = prepare_in_maps(
        x, qkv_w, qkv_b, proj_w, proj_b, rel_pos_h, rel_pos_w)
    if _NC is None:
        _NC = build()

    res = run_bass_kernel_spmd(_NC, in_maps, core_ids=list(range(NCORES)))
    LAST_EXEC_NS = getattr(res, "exec_time_ns", None)
    halves = []
    for qh in range(NQH):
        acc = res.results[qh * NHG + 0]["out"].astype(np.float64)
        for hg in range(1, NHG):
            acc = acc + res.results[qh * NHG + hg]["out"]
        halves.append(acc.T)
    y = np.concatenate(halves, axis=0).reshape(1, 64, 64, C)
    return np.ascontiguousarray(y, dtype=np.float32)


# revision 32
# speedup vs baseline: 1.1551x; 1.1551x over previous
"""Distributed Trainium2 kernel for ViTDet-style global attention with
decomposed relative position bias (B=1, H=W=64, C=768, 12 heads, hd=64).

Sharding: 4 head-groups x 2 query-halves over 8 cores. Core r handles
heads 3*hg..3*hg+2 (hg = r % 4) and queries qh*2048..qh*2048+2047
(qh = r // 4). Each core computes K/V for its 3 heads over all 4096
keys, attention for its 2048 queries, and a PARTIAL output projection
contracting only its 192 channels; the host sums the 4 partials per
query half. No collectives.

Inside a core the 3x2048 work is organized as 12 "virtual heads"
vh = qs*3 + h of (512-query subtile, head), reusing the proven
single-core structure: rel_h folded into the QK matmul via one-hot
indicator rows on an augmented K, rel_w applied as a post-exp
multiplicative factor (its partition pattern has period 64, matching
k mod 64), softmax without max subtraction, PV with a ones-column for
the denominator.

Scheduling: a minimal pre-attention phase (K heads 0/1, V heads 0/1,
q-projection + rel tables for query-subtile 0) starts attention ~30us
in; K head 2, V head 2, the qs1-3 q/rel chains, softmax denominators,
and the output projection are all deferred and pumped through the
attention phase's PE slack via a deadline-tagged fill queue, so the
Act engine (exp) runs back-to-back. Startup DMAs are split across the
SP and Act DMA queues.
"""

import sys

import numpy as np
import ml_dtypes

for p in ("/opt/trn_rl_repo",):
    if p not in sys.path:
        sys.path.insert(0, p)

import concourse.mybir as mybir
from concourse import bacc
from concourse.tile import TileContext
from concourse.bass_utils import run_bass_kernel_spmd

NCORES = 8
S, C, NH, HD = 4096, 768, 12, 64
NHG = 4                   # head groups
NQH = 2                   # query halves
HPC = NH // NHG           # 3 heads per core
SQH = S // NQH            # 2048 queries per core
NVH = HPC * 4             # 12 virtual heads (512-q subtile, head)
SQ = 512                  # queries per virtual head
HLOC = SQH // 64          # 32 h-rows per core
BLK = 512                 # x block (keys)
NBLK = S // BLK           # 8
NKC = S // 128            # 32 key chunks
EBATCH = 2                # logits chunks per exp batch (2 PSUM banks)
CH = HPC * HD             # 192 channels per core
F32 = mybir.dt.float32
BF = mybir.dt.bfloat16
Exp = mybir.ActivationFunctionType.Exp
Ident = mybir.ActivationFunctionType.Identity
Mult = mybir.AluOpType.mult
BF_NP = ml_dtypes.bfloat16

LAST_EXEC_NS = None
DEBUG_DUMPS = False


def build():
    nc = bacc.Bacc(None, target_bir_lowering=False)

    xT = nc.dram_tensor("xT", (6, 128, S), BF, kind="ExternalInput")
    wq = nc.dram_tensor("wq", (128, 6, CH), BF, kind="ExternalInput")
    wk = nc.dram_tensor("wk", (128, 6, CH), BF, kind="ExternalInput")
    wv = nc.dram_tensor("wv", (128, 6, CH), BF, kind="ExternalInput")
    pwA = nc.dram_tensor("pwA", (128, 6, 128), BF, kind="ExternalInput")
    pwB = nc.dram_tensor("pwB", (64, 6, 128), BF, kind="ExternalInput")
    bq = nc.dram_tensor("bq", (64, HPC), F32, kind="ExternalInput")
    pb2 = nc.dram_tensor("pb2", (128, 6), F32, kind="ExternalInput")
    rhT = nc.dram_tensor("rhT", (HD, HLOC * 64), BF, kind="ExternalInput")
    rwT = nc.dram_tensor("rwT", (HD, 64 * 64), BF, kind="ExternalInput")
    idm = nc.dram_tensor("idm", (128, 128), BF, kind="ExternalInput")
    khfull = nc.dram_tensor("khfull", (64, S), BF, kind="ExternalInput")
    out = nc.dram_tensor("out", (C, SQH), F32, kind="ExternalOutput")

    with TileContext(nc) as tc:
        with (
            nc.allow_low_precision(reason="bf16 matmul inputs"),
            tc.tile_pool(name="per", bufs=1) as per,
            tc.tile_pool(name="pt", bufs=6) as ptp,
            tc.tile_pool(name="ysb", bufs=2) as ysbp,
            tc.tile_pool(name="sml", bufs=1) as smlp,
            tc.tile_pool(name="stp", bufs=3, space="PSUM") as stp,
            tc.tile_pool(name="pvp", bufs=2, space="PSUM") as pvp,
        ):
            # ---- DMAs on the SP queue: pb, K/V weights, x blocks ----
            pb_sb = per.tile([128, 6], F32, tag="pbsb")
            nc.sync.dma_start(pb_sb[:], pb2[:])
            idt = per.tile([128, 128], BF, tag="idt")
            nc.sync.dma_start(idt[:], idm[:])
            wk_sb = per.tile([128, 6, CH], BF, tag="wk")
            nc.sync.dma_start(wk_sb[:], wk[:])
            wv_sb = per.tile([128, 6, CH], BF, tag="wv")
            nc.sync.dma_start(wv_sb[:], wv[:])
            xall = per.tile([128, 6, S], BF, tag="xall")
            for blk in range(NBLK):
                sl = slice(blk * BLK, (blk + 1) * BLK)
                eng = nc.sync if blk % 2 == 0 else nc.scalar
                eng.dma_start(
                    xall[:, :, sl], xT[:, :, sl].transpose([1, 0, 2]))

            # ---- DMAs on the Act queue (parallel with the above) ----
            bq_sb = per.tile([64, HPC], F32, tag="bq")
            nc.scalar.dma_start(bq_sb[:], bq[:])
            wq_sb = per.tile([128, 6, CH], BF, tag="wq")
            nc.scalar.dma_start(wq_sb[:], wq[:])
            rhT_sb = per.tile([HD, HLOC * 64], BF, tag="rhT")
            nc.scalar.dma_start(rhT_sb[:], rhT[:])
            rwT_sb = per.tile([HD, 64 * 64], BF, tag="rwT")
            nc.scalar.dma_start(rwT_sb[:], rwT[:])

            # ---- persistent tiles ----
            kts = [per.tile([128, S], BF, tag=f"kt{i}", name=f"kt{i}")
                   for i in range(HPC)]
            for i in range(HPC):
                nc.scalar.dma_start(kts[i][64:128, :], khfull[:])
            pwA_sb = per.tile([128, 6, 128], BF, tag="pwA")
            nc.scalar.dma_start(pwA_sb[:], pwA[:])
            pwB_sb = per.tile([64, 6, 128], BF, tag="pwB")
            nc.scalar.dma_start(pwB_sb[:], pwB[:])

            vtA = per.tile([128, NKC, HPC, 66], BF, tag="vtA")
            nc.vector.memset(vtA[:, :, :, 64], 1.0)
            qaT = per.tile([128, NVH, SQ], BF, tag="qaT")
            relwA = per.tile([64, NVH, SQ], BF, tag="relwA")
            ewA = per.tile([128, NVH, SQ], BF, tag="ewA")
            aout0 = per.tile([128, SQH], BF, tag="ao0")
            aout1 = per.tile([64, SQH], BF, tag="ao1")

            # ---- emit helpers ----
            def kpair_blk(blk):
                # K for heads 0,1 packed on 128 partitions
                sl = slice(blk * BLK, (blk + 1) * BLK)
                psk0 = stp.tile([128, 2, 512], F32, tag="st")
                psk = psk0[:, 0, :]
                for cc in range(6):
                    nc.tensor.matmul(
                        psk[:], wk_sb[:, cc, 0:128], xall[:, cc, sl],
                        start=(cc == 0), stop=(cc == 5),
                        skip_group_check=True)
                nc.scalar.copy(kts[0][0:64, sl], psk[0:64, :])
                nc.vector.tensor_copy(kts[1][0:64, sl], psk[64:128, :])

            def v01_blk(blk):
                # V for heads 0,1 per 128-key chunk
                for rb in range(4):
                    kc = 4 * blk + rb
                    ksl = slice(blk * BLK + rb * 128,
                                blk * BLK + (rb + 1) * 128)
                    psv0 = stp.tile([128, 16, 64], F32, tag="st")
                    psv = psv0[:, 0:2, :]
                    for cc in range(6):
                        nc.tensor.matmul(
                            psv[:], xall[:, cc, ksl], wv_sb[:, cc, 0:128],
                            start=(cc == 0), stop=(cc == 5),
                            skip_group_check=True)
                    nc.vector.tensor_copy(vtA[:, kc, 0:2, 0:64], psv[:])

            def k2_blk(blk):
                # K head 2, one 512-key block ([64,512] out) per item
                sl = slice(blk * BLK, (blk + 1) * BLK)
                ps0 = stp.tile([128, 2, 512], F32, tag="st")
                ps2 = ps0[0:64, 0, :]
                for cc in range(6):
                    nc.tensor.matmul(
                        ps2[:], wk_sb[:, cc, 128:192], xall[:, cc, sl],
                        start=(cc == 0), stop=(cc == 5),
                        skip_group_check=True)
                nc.vector.tensor_copy(kts[2][0:64, sl], ps2[:])

            def v2_rb(kc):
                ksl = slice(kc * 128, (kc + 1) * 128)
                psv0 = stp.tile([128, 16, 64], F32, tag="st")
                psv = psv0[:, 0, :]
                for cc in range(6):
                    nc.tensor.matmul(
                        psv[:], xall[:, cc, ksl], wv_sb[:, cc, 128:192],
                        start=(cc == 0), stop=(cc == 5),
                        skip_group_check=True)
                nc.vector.tensor_copy(vtA[:, kc, 2, 0:64], psv[:])

            def qproj(qs):
                # heads 0,1 packed + head 2; bias folded into the copies.
                # Queries are xall columns 0:2048 (keys pre-rolled per
                # core so each core's query window sits at offset 0).
                ps0 = stp.tile([128, 2, 512], F32, tag="st")
                ps = ps0[:, 0, :]
                qsl = slice(qs * 512, (qs + 1) * 512)
                for cc in range(6):
                    nc.tensor.matmul(
                        ps[:], wq_sb[:, cc, 0:128], xall[:, cc, qsl],
                        start=(cc == 0), stop=(cc == 5),
                        skip_group_check=True)
                ps2 = ps0[0:64, 1, :]
                for cc in range(6):
                    nc.tensor.matmul(
                        ps2[:], wq_sb[:, cc, 128:192], xall[:, cc, qsl],
                        start=(cc == 0), stop=(cc == 5),
                        skip_group_check=True)
                for hh in range(2):
                    nc.scalar.activation(
                        qaT[0:64, qs * 3 + hh, :],
                        ps[hh * 64:hh * 64 + 64, :],
                        Ident, bias=bq_sb[:, hh:hh + 1])
                nc.scalar.activation(
                    qaT[0:64, qs * 3 + 2, :], ps2[:],
                    Ident, bias=bq_sb[:, 2:3])

            def relh(b):
                qs = b // 3
                ps0 = stp.tile([128, 2, 512], F32, tag="st")
                ps = ps0[0:64, 0, :]
                for hl in range(8):
                    lr = qs * 8 + hl
                    nc.tensor.matmul(
                        ps[:, hl * 64:(hl + 1) * 64],
                        rhT_sb[:, lr * 64:(lr + 1) * 64],
                        qaT[0:64, b, hl * 64:(hl + 1) * 64],
                        start=True, stop=True, skip_group_check=True)
                nc.vector.tensor_copy(qaT[64:128, b, :], ps[:])

            def relw(wv0, n):
                # one matmul per w-coordinate covers all 12 virtual heads
                for wvi in range(wv0, wv0 + n):
                    ps0 = stp.tile([128, 4, 12, 8], F32, tag="st")
                    ps = ps0[0:64, 0, :, :]
                    nc.tensor.matmul(
                        ps[:],
                        rwT_sb[:, wvi * 64:(wvi + 1) * 64],
                        qaT[0:64, :, wvi:wvi + 449:64],
                        start=True, stop=True, skip_group_check=True)
                    if wvi % 2 == 0:
                        nc.vector.tensor_copy(
                            relwA[:, :, wvi:wvi + 449:64], ps[:])
                    else:
                        nc.scalar.copy(
                            relwA[:, :, wvi:wvi + 449:64], ps[:])

            def ew_exp():
                nc.scalar.activation(ewA[0:64, :, :], relwA[:], Exp)
                nc.vector.tensor_copy(ewA[64:128, :, :], ewA[0:64, :, :])

            def out_proj(t, qs):
                ps0 = stp.tile([128, 2, 512], F32, tag="st")
                ps = ps0[:, 0, :]
                sl = slice(qs * 512, (qs + 1) * 512)
                nc.tensor.matmul(
                    ps[:], pwA_sb[:, t, :], aout0[:, sl],
                    start=True, stop=False, skip_group_check=True)
                nc.tensor.matmul(
                    ps[:], pwB_sb[:, t, :], aout1[:, sl],
                    start=False, stop=True, skip_group_check=True)
                y = ysbp.tile([128, 512], F32, tag="y")
                nc.vector.tensor_scalar_add(y[:], ps[:], pb_sb[:, t:t + 1])
                nc.sync.dma_start(out[t * 128:(t + 1) * 128, sl], y[:])

            # ---- pre-attention phase ----
            for blk in range(NBLK):
                kpair_blk(blk)
                v01_blk(blk)
                if blk >= 1 and blk <= 4:
                    qproj(blk - 1)
                if blk >= 5:
                    relw((blk - 5) * 16, 16)
            relw(48, 16)
            for b in range(3):
                relh(b)
            ew_exp()

            # ---- deferred work: deadline-tagged fill queue ----
            fill_q = []

            def enqueue(deadline, fn):
                fill_q.append((deadline, fn))

            for blk in range(NBLK):
                enqueue(2, lambda blk=blk: k2_blk(blk))
                enqueue(2, lambda kc=4 * blk: v2_rb(kc))
                enqueue(2, lambda kc=4 * blk + 1: v2_rb(kc))
                enqueue(2, lambda kc=4 * blk + 2: v2_rb(kc))
                enqueue(2, lambda kc=4 * blk + 3: v2_rb(kc))
            for b in range(3, NVH):
                enqueue(b, lambda b=b: relh(b))

            def pump(n):
                for _ in range(n):
                    if fill_q:
                        fill_q.pop(0)[1]()

            def drain_until(dl):
                while fill_q and fill_q[0][0] <= dl:
                    fill_q.pop(0)[1]()

            # ---- attention, Act-saturated pipeline ----
            batches = []
            kc = 0
            while kc < NKC:
                nb = min(EBATCH, NKC - kc)
                batches.append((kc, nb))
                kc += nb
            NB = len(batches)

            def qk_batch(b, kc, nb):
                st = stp.tile([128, 2, 512], F32, tag="st")
                kt = kts[b % 3]
                for sub in range(nb):
                    nc.tensor.matmul(
                        st[:, sub, :],
                        kt[:, (kc + sub) * 128:(kc + sub + 1) * 128],
                        qaT[:, b, :],
                        start=True, stop=True, skip_group_check=True)
                return st

            def make_denoms(b, pv):
                # 5 thunks: reciprocal, then per-qc scale+transpose+copy;
                # the last thunk enqueues the output projections that
                # become ready once this vh's aout columns are complete.
                head, qs = b % 3, b // 3
                state = {}

                def d_recip():
                    r = smlp.tile([128, 4, 1], F32, tag="recip", bufs=2)
                    nc.vector.reciprocal(r[:], pv[:, :, 64:65])
                    state["r"] = r

                def d_qc(qc):
                    aoT = smlp.tile([128, 64], BF, tag="aoT", bufs=2)
                    nc.vector.tensor_scalar_mul(
                        aoT[:], pv[:, qc, 0:64], state["r"][:, qc, :])
                    tps = pv[0:64, qc, 64:128].bitcast(BF)
                    nc.tensor.transpose(tps, aoT[:], idt[:])
                    dst_q = qs * 512 + qc * 128
                    if head < 2:
                        nc.vector.tensor_copy(
                            aout0[head * 64:head * 64 + 64,
                                  dst_q:dst_q + 128], tps)
                    else:
                        nc.vector.tensor_copy(
                            aout1[0:64, dst_q:dst_q + 128], tps)
                    if head == 2 and qc == 3:
                        for t in range(6):
                            enqueue(99, lambda t=t, qs=qs: out_proj(t, qs))

                return [d_recip] + [lambda qc=qc: d_qc(qc)
                                    for qc in range(4)]

            st = qk_batch(0, *batches[0])
            denom_prev = []
            for b in range(NVH):
                pv = pvp.tile([128, 4, 128], F32, tag="pv")
                hh = b % 3
                for bi, (kc, nb) in enumerate(batches):
                    pt = ptp.tile([128, 2, 512], BF, tag="pt")
                    nc.scalar.activation(
                        pt[:, 0:nb, :], st[:, 0:nb, :], Exp)
                    nc.vector.tensor_tensor(
                        pt[:, 0:nb, :], pt[:, 0:nb, :],
                        ewA[:, b, :].unsqueeze(1).to_broadcast(
                            (128, nb, 512)),
                        Mult)
                    # next QK before this PV so PE stays a batch ahead
                    if bi + 1 < NB:
                        st = qk_batch(b, *batches[bi + 1])
                    elif b + 1 < NVH:
                        drain_until(b + 1)
                        st = qk_batch(b + 1, *batches[0])
                    for sub in range(nb):
                        for qc in range(4):
                            nc.tensor.matmul(
                                pv[:, qc, 0:65],
                                pt[:, sub, qc * 128:(qc + 1) * 128],
                                vtA[:, kc + sub, hh, 0:65],
                                start=(kc + sub == 0 and qc == 0),
                                stop=(kc + sub == NKC - 1 and qc == 3),
                                skip_group_check=True)
                    if bi < len(denom_prev):
                        denom_prev[bi]()
                    pump(2 if len(fill_q) > 24 else 1)
                denom_prev = make_denoms(b, pv)

            for fn in denom_prev:
                fn()
            pump(len(fill_q))

            if DEBUG_DUMPS:
                d_qaT = nc.dram_tensor(
                    "d_qaT", (128, NVH, SQ), BF, kind="ExternalOutput")
                nc.sync.dma_start(d_qaT[:], qaT[:])
                d_kt0 = nc.dram_tensor(
                    "d_kt0", (128, S), BF, kind="ExternalOutput")
                nc.sync.dma_start(d_kt0[:], kts[0][:])
                d_vt = nc.dram_tensor(
                    "d_vt", (128, NKC, HPC, 65), BF, kind="ExternalOutput")
                nc.sync.dma_start(d_vt[:], vtA[:, :, :, 0:65])
                d_ew = nc.dram_tensor(
                    "d_ew", (128, NVH, SQ), BF, kind="ExternalOutput")
                nc.sync.dma_start(d_ew[:], ewA[:])
                d_ao0 = nc.dram_tensor(
                    "d_ao0", (128, SQH), BF, kind="ExternalOutput")
                nc.sync.dma_start(d_ao0[:], aout0[:])
                d_ao1 = nc.dram_tensor(
                    "d_ao1", (64, SQH), BF, kind="ExternalOutput")
                nc.sync.dma_start(d_ao1[:], aout1[:])

    nc.compile()
    return nc


def prepare_in_maps(x, qkv_w, qkv_b, proj_w, proj_b, rel_pos_h, rel_pos_w):
    x = np.asarray(x, np.float32)
    qkv_w = np.asarray(qkv_w, np.float32)
    qkv_b = np.asarray(qkv_b, np.float32)
    proj_w = np.asarray(proj_w, np.float32)
    proj_b = np.asarray(proj_b, np.float32)
    rel_pos_h = np.asarray(rel_pos_h, np.float32)
    rel_pos_w = np.asarray(rel_pos_w, np.float32)

    scale = HD ** -0.5
    xT = np.ascontiguousarray(x.reshape(S, C).T)          # (768, 4096)
    xTr = xT.reshape(6, 128, S).astype(BF_NP)

    w_q_full = (qkv_w[:, :C] * scale).reshape(6, 128, C)
    w_k_full = qkv_w[:, C:2 * C].reshape(6, 128, C)
    w_v_full = qkv_w[:, 2 * C:].reshape(6, 128, C)
    bq_full = (qkv_b[:C] * scale)
    bv = qkv_b[2 * C:]

    k_idx = np.arange(64)
    rwT = np.empty((HD, 64, 64), np.float32)
    for wvi in range(64):
        rwT[:, wvi, :] = rel_pos_w[wvi + 63 - k_idx, :].T / scale
    rwT = rwT.reshape(HD, 64 * 64).astype(BF_NP)

    idmn = np.eye(128, dtype=np.float32).astype(BF_NP)
    khfull = ((np.arange(S)[None, :] // 64 == np.arange(64)[:, None])
              .astype(BF_NP))
    # Per-query-half KEY ROLL: softmax is permutation-invariant over
    # keys, and the rel_w partition pattern has period 64 (2048 % 64
    # == 0), so rolling the key axis by -qh*2048 lets every core read
    # its own query window at xall columns 0:2048. khfull (the h-row
    # indicator) must be rolled consistently.
    xTr_r = [np.roll(xTr, -qh * SQH, axis=2) for qh in range(NQH)]
    khfull_r = [np.roll(khfull, -qh * SQH, axis=1) for qh in range(NQH)]

    rhTs = []
    for qh in range(NQH):
        rhm = np.empty((HD, HLOC, 64), np.float32)
        for lr in range(HLOC):
            h = qh * HLOC + lr
            rhm[:, lr, :] = rel_pos_h[h + 63 - k_idx, :].T / scale
        rhTs.append(rhm.reshape(HD, HLOC * 64).astype(BF_NP))

    wqs, wks, wvs, pwAs, pwBs, bqs, pb2s = [], [], [], [], [], [], []
    for hg in range(NHG):
        cs = slice(hg * CH, (hg + 1) * CH)
        wqs.append(np.ascontiguousarray(
            w_q_full[:, :, cs].transpose(1, 0, 2)).astype(BF_NP))
        wks.append(np.ascontiguousarray(
            w_k_full[:, :, cs].transpose(1, 0, 2)).astype(BF_NP))
        wvs.append(np.ascontiguousarray(
            w_v_full[:, :, cs].transpose(1, 0, 2)).astype(BF_NP))
        pw_slice = proj_w[hg * CH:(hg + 1) * CH, :]     # (192, 768)
        pwAs.append(np.ascontiguousarray(
            pw_slice[0:128].reshape(128, 6, 128)).astype(BF_NP))
        pwBs.append(np.ascontiguousarray(
            pw_slice[128:192].reshape(64, 6, 128)).astype(BF_NP))
        bqs.append(np.ascontiguousarray(
            bq_full[cs].reshape(HPC, 64).T).astype(np.float32))
        pb_r = bv[cs] @ pw_slice
        if hg == 0:
            pb_r = pb_r + proj_b
        pb2s.append(np.ascontiguousarray(
            pb_r.reshape(6, 128).T).astype(np.float32))

    in_maps = []
    for r in range(NCORES):
        hg, qh = r % NHG, r // NHG
        in_maps.append({
            "xT": xTr_r[qh], "wq": wqs[hg], "wk": wks[hg],
            "wv": wvs[hg], "pwA": pwAs[hg], "pwB": pwBs[hg],
            "bq": bqs[hg], "pb2": pb2s[hg], "rhT": rhTs[qh], "rwT": rwT,
            "idm": idmn, "khfull": khfull_r[qh],
        })
    return in_maps


_NC = None


def kernel(x, qkv_w, qkv_b, proj_w, proj_b, rel_pos_h, rel_pos_w):
    global _NC, LAST_EXEC_NS
    in_maps = prepare_in_maps(
        x, qkv_w, qkv_b, proj_w, proj_b, rel_pos_h, rel_pos_w)
    if _NC is None:
        _NC = build()

    res = run_bass_kernel_spmd(_NC, in_maps, core_ids=list(range(NCORES)))
    LAST_EXEC_NS = getattr(res, "exec_time_ns", None)
    halves = []
    for qh in range(NQH):
        acc = res.results[qh * NHG + 0]["out"].astype(np.float64)
        for hg in range(1, NHG):
            acc = acc + res.results[qh * NHG + hg]["out"]
        halves.append(acc.T)
    y = np.concatenate(halves, axis=0).reshape(1, 64, 64, C)
    return np.ascontiguousarray(y, dtype=np.float32)
